# revision 9
# baseline (speedup 1.0000x reference)
"""Multi-head attention (B=2, T=2048, D=2048, 16 heads) on 8 NeuronCores.

Sharding: DP=2 over batch x TP=4 over heads (4 heads/core).
Core c handles batch b=c//4, head group r=c%4 (heads 4r..4r+3).

Per-core dataflow (all matmuls in float32r, single-pass FP22 on PE):
  P1: Q^T, K^T (dh-on-partitions) and V (tokens-on-partitions) projections.
      Host passes x[b]^T and W^T slices so every matmul operand is in its
      natural layout -- no on-device transposes anywhere.
  P2: per head: S^T = K_h^T^T@Q_h^T chunks -> exp (ScalarE, scaled 1/sqrt(dh))
      -> PV accumulation (attn^T in PSUM) with column sums via a ones-matmul;
      normalize with DVE using a DMA-broadcast reciprocal.
  P3: AllGather attn^T over the 4-core batch group, then each core computes
      its 512 output columns: out = attn_full @ Wo^T[:, cols].

Output per core: (2048 tokens, 512 out-cols); host concatenates.
"""

import math

import numpy as np

import concourse.bass as bass
import concourse.mybir as mybir
import concourse.tile as tile
from concourse import bacc
from concourse.bass_utils import run_bass_kernel_spmd

D = 2048
T = 2048
HG = 4  # heads per core
DH = 128
NI = 16  # contraction chunks of 128 over D
NQ = 4  # query-token chunks of 512
NT = 16  # token chunks of 128
SCALE = 1.0 / math.sqrt(DH)
F32 = mybir.dt.float32
F32R = mybir.dt.float32r
GROUPS = [[0, 1, 2, 3], [4, 5, 6, 7]]

_CACHED = {}


def build():
    nc = bacc.Bacc("TRN2", target_bir_lowering=False, debug=False, num_devices=8)
    xT = nc.declare_dram_parameter("xT", [D, T], F32R, isOutput=False)
    wqT = nc.declare_dram_parameter("wqT", [D, HG * DH], F32R, isOutput=False)
    wkT = nc.declare_dram_parameter("wkT", [D, HG * DH], F32R, isOutput=False)
    wvT = nc.declare_dram_parameter("wvT", [D, HG * DH], F32R, isOutput=False)
    woT = nc.declare_dram_parameter("woT", [D, HG * DH], F32R, isOutput=False)
    out = nc.declare_dram_parameter("out", [T, HG * DH], F32, isOutput=True)

    with tile.TileContext(nc) as tc:
        with (
            tc.tile_pool(name="dram", bufs=1, space="DRAM") as dram,
            tc.tile_pool(name="keep", bufs=1) as keep,
        ):
            attn_mine = dram.tile([HG * DH, T], F32R)
            attn_all = dram.tile([4 * HG * DH, T], F32R)
            qT_d = dram.tile([HG * DH, T], F32R)
            kT_d = dram.tile([HG * DH, T], F32R)

            v_sb = keep.tile([128, NT, HG * DH], F32R)  # V: [tok128, tchunk, hdims]
            ones_f32 = keep.tile([128, 1], F32)
            nc.vector.memset(ones_f32[:], 1.0)
            ones_sb = keep.tile([128, 1], F32R)
            nc.vector.tensor_copy(ones_sb[:], ones_f32[:])

            # ---------------- Phase 1: QKV projections ----------------
            with (
                tc.tile_pool(name="p1x", bufs=1) as p1x,
                tc.tile_pool(name="p1w", bufs=1) as p1w,
                tc.tile_pool(name="p1s", bufs=6) as p1s,
                tc.tile_pool(name="p1p", bufs=4, space="PSUM") as p1p,
            ):
                x_sb = p1x.tile([128, NI, T], F32R)  # x^T resident: 128KB/part
                for i in range(NI):
                    nc.sync.dma_start(
                        out=x_sb[:, i, :], in_=xT[i * 128 : (i + 1) * 128, :]
                    )

                # Q^T and K^T: out rows = head dims (M), moving = tokens
                for w_par, dst in ((wqT, qT_d), (wkT, kT_d)):
                    w_sb = p1w.tile([128, NI, HG * DH], F32R, tag="w_sb")
                    for i in range(NI):
                        nc.sync.dma_start(
                            out=w_sb[:, i, :], in_=w_par[i * 128 : (i + 1) * 128, :]
                        )
                    for m in range(HG):
                        psums = []
                        for t in range(NQ):
                            psums.append(
                                p1p.tile([128, 512], F32, name="qk_ps", tag="qk_ps")
                            )
                        for i in range(NI):
                            lhsT = w_sb[:, i, m * 128 : (m + 1) * 128]
                            for t in range(NQ):
                                nc.tensor.matmul(
                                    psums[t][:],
                                    lhsT,
                                    x_sb[:, i, t * 512 : (t + 1) * 512],
                                    start=(i == 0),
                                    stop=(i == NI - 1),
                                )
                        for t in range(NQ):
                            st = p1s.tile([128, 512], F32R)
                            nc.vector.tensor_copy(st[:], psums[t][:])
                            nc.sync.dma_start(
                                out=dst[
                                    m * 128 : (m + 1) * 128, t * 512 : (t + 1) * 512
                                ],
                                in_=st[:],
                            )

                # V: natural layout, tokens = M (stationary = x^T chunk)
                w_sb = p1w.tile([128, NI, HG * DH], F32R, tag="w_sb")
                for i in range(NI):
                    nc.sync.dma_start(
                        out=w_sb[:, i, :], in_=wvT[i * 128 : (i + 1) * 128, :]
                    )
                for tc_i in range(NT):
                    ps = p1p.tile([128, 512], F32)
                    for i in range(NI):
                        nc.tensor.matmul(
                            ps[:],
                            x_sb[:, i, tc_i * 128 : (tc_i + 1) * 128],
                            w_sb[:, i, :],
                            start=(i == 0),
                            stop=(i == NI - 1),
                        )
                    nc.vector.tensor_copy(v_sb[:, tc_i, :], ps[:])

            # ---------------- Phase 2: attention per head ----------------
            with (
                tc.tile_pool(name="p2qk", bufs=2) as p2qk,
                tc.tile_pool(name="p2e", bufs=4) as p2e,
                tc.tile_pool(name="p2a", bufs=2) as p2a,
                tc.tile_pool(name="p2n", bufs=2) as p2n,
                tc.tile_pool(name="p2ps", bufs=3, space="PSUM") as p2ps,
                tc.tile_pool(name="p2pa", bufs=2, space="PSUM") as p2pa,
                tc.tile_pool(name="p2pc", bufs=2, space="PSUM") as p2pc,
            ):
                for h in range(HG):
                    qh = p2qk.tile([128, T], F32R, tag="qh")
                    kh = p2qk.tile([128, T], F32R, tag="kh")
                    nc.sync.dma_start(out=qh[:], in_=qT_d[h * 128 : (h + 1) * 128, :])
                    nc.sync.dma_start(out=kh[:], in_=kT_d[h * 128 : (h + 1) * 128, :])
                    for q in range(NQ):
                        acc = p2a.tile([128, 512], F32R, tag="acc")
                        attn_ps = p2pa.tile([128, 512], F32, tag="attn_ps")
                        for k in range(NT):
                            s_ps = p2ps.tile([128, 512], F32, tag="s_ps")
                            nc.tensor.matmul(
                                s_ps[:],
                                kh[:, k * 128 : (k + 1) * 128],
                                qh[:, q * 512 : (q + 1) * 512],
                            )
                            expS = p2e.tile([128, 512], F32R, tag="expS")
                            nc.scalar.activation(
                                expS[:],
                                s_ps[:],
                                mybir.ActivationFunctionType.Exp,
                                scale=SCALE,
                            )
                            if k == 0:
                                nc.vector.tensor_copy(acc[:], expS[:])
                            else:
                                nc.vector.tensor_add(acc[:], acc[:], expS[:])
                            nc.tensor.matmul(
                                attn_ps[:],
                                v_sb[:, k, h * 128 : (h + 1) * 128],
                                expS[:],
                                start=(k == 0),
                                stop=(k == NT - 1),
                            )
                        csum = p2pc.tile([1, 512], F32, tag="csum")
                        nc.tensor.matmul(
                            csum[:], ones_sb[:], acc[:]
                        )
                        recip = p2n.tile([1, 512], F32, tag="recip")
                        nc.vector.reciprocal(recip[:], csum[:])
                        recip_d = dram.tile(
                            [1, 512], F32, name="recip_d", tag="recip_d", bufs=2
                        )
                        nc.sync.dma_start(out=recip_d[:], in_=recip[:])
                        bc = p2n.tile([128, 512], F32, tag="bc")
                        bcast_src = bass.AP(
                            tensor=recip_d.tensor,
                            offset=recip_d.offset,
                            ap=[[0, 128]] + [list(x) for x in recip_d.ap[1:]],
                        )
                        nc.sync.dma_start(out=bc[:], in_=bcast_src)
                        attn_sb = p2a.tile([128, 512], F32R, tag="attn_sb")
                        nc.vector.tensor_mul(attn_sb[:], attn_ps[:], bc[:])
                        nc.sync.dma_start(
                            out=attn_mine[
                                h * 128 : (h + 1) * 128, q * 512 : (q + 1) * 512
                            ],
                            in_=attn_sb[:],
                        )

            # ---------------- AllGather over batch group ----------------
            nc.gpsimd.collective_compute(
                "AllGather",
                mybir.AluOpType.bypass,
                replica_groups=GROUPS,
                ins=[attn_mine.opt()],
                outs=[attn_all.opt()],
            )

            # ---------------- Phase 3: output projection ----------------
            with (
                tc.tile_pool(name="p3w", bufs=1) as p3w,
                tc.tile_pool(name="p3a", bufs=8) as p3a,
                tc.tile_pool(name="p3o", bufs=4) as p3o,
                tc.tile_pool(name="p3p", bufs=4, space="PSUM") as p3p,
            ):
                wo_sb = p3w.tile([128, NI, HG * DH], F32R)
                for i in range(NI):
                    nc.sync.dma_start(
                        out=wo_sb[:, i, :], in_=woT[i * 128 : (i + 1) * 128, :]
                    )
                for t in range(NT):
                    ps = p3p.tile([128, 512], F32)
                    for i in range(NI):
                        a_tile = p3a.tile([128, 128], F32R, tag="a_tile")
                        nc.sync.dma_start(
                            out=a_tile[:],
                            in_=attn_all[
                                i * 128 : (i + 1) * 128, t * 128 : (t + 1) * 128
                            ],
                        )
                        nc.tensor.matmul(
                            ps[:],
                            a_tile[:],
                            wo_sb[:, i, :],
                            start=(i == 0),
                            stop=(i == NI - 1),
                        )
                    o_sb = p3o.tile([128, 512], F32)
                    nc.vector.tensor_copy(o_sb[:], ps[:])
                    nc.sync.dma_start(
                        out=out[t * 128 : (t + 1) * 128, :], in_=o_sb[:]
                    )

    nc.compile()
    return nc


def _get_nc():
    if "nc" not in _CACHED:
        _CACHED["nc"] = build()
    return _CACHED["nc"]


def kernel(x, Wq, Wk, Wv, Wo, _trace=False):
    x = np.asarray(x, dtype=np.float32)
    Wq = np.asarray(Wq, dtype=np.float32)
    Wk = np.asarray(Wk, dtype=np.float32)
    Wv = np.asarray(Wv, dtype=np.float32)
    Wo = np.asarray(Wo, dtype=np.float32)
    B = x.shape[0]

    in_maps = []
    for c in range(8):
        b, r = divmod(c, 4)
        sl = slice(r * 512, (r + 1) * 512)
        in_maps.append(
            {
                "xT": np.ascontiguousarray(x[b].T),
                "wqT": np.ascontiguousarray(Wq[sl, :].T),
                "wkT": np.ascontiguousarray(Wk[sl, :].T),
                "wvT": np.ascontiguousarray(Wv[sl, :].T),
                "woT": np.ascontiguousarray(Wo[sl, :].T),
            }
        )

    nc = _get_nc()
    res = run_bass_kernel_spmd(nc, in_maps, list(range(8)), trace=_trace)
    _CACHED["last_result"] = res

    out = np.empty((B, T, D), dtype=np.float32)
    for c in range(8):
        b, r = divmod(c, 4)
        out[b, :, r * 512 : (r + 1) * 512] = res.results[c]["out"]
    return out
